# revision 1
# baseline (speedup 1.0000x reference)
"""CrossLinearAttention Trainium2 kernel (fp16 matmul v2).

Per-core: one batch sample (data-parallel over B=8 across 8 NeuronCores).
Per sample x_b: [C=128, N=65536] fp32, shipped as fp16 hi/lo pair.

Math (matches the reference exactly, re-associated for TRN2):
  q,k,v = W{q,k,v} @ x ; q softmaxed over d_head (32-groups), k over n.
  ctx_h = softmax_k_h @ v_h^T  (32x32/head) ; out2 = ctx^T q * SCALE
  out = Wo@out2 + bo ; GroupNorm(1 group) over (C,N) ; *gamma+beta ; +x

Passes (all big matmuls in fp16 = 1 PE cycle/row; fp32 PSUM accumulate):
  pass1: per 128-col chunk, kvT = x^T @ [Wk^T|Wv^T]; ek=exp(kT) (fp16);
         PSUM ctxz += ek^T @ [vT | 1]  ->  E V^T (full) and Z.
  mid1 : ctxN = (ctx/Z).*blockmask ; W1T' = ctxN @ (Wo^T*SCALE*4096) (fp16)
  pass2: qraw=Wq@x ; expq=exp(qraw) ; per 4-chunk group the head-denoms
         D go into one PSUM tile at partition offsets {0,32,64,96};
         rd = approx_recip(D) (one DVE op per group) -> fp16;
         rb = bcast_mm(rd) ; qn = expq*rb (DVE TTR, accum -> sum-qn slots);
         out' = W1T'^T @ qn ; ACT Square(accum) -> sum-sq slots.
  mid2 : GroupNorm stats from S1=W1T'^T(sum qn), S2=sum-sq (bias folded
         analytically, 4096 scaling undone); rstd=exp(-0.5 ln(V+eps));
         W2T = W1T' * (rstd*gamma/4096) ; s2 = (bo-mean)*rstd*gamma+beta.
  pass3: recompute qn; outF = W2T^T @ qn; final = outF+s2+x_hi (+x_lo on
         GpSimd); store fp32.
"""

import sys

sys.path.insert(0, "/opt/trn_rl_repo")

import functools
from contextlib import ExitStack

import numpy as np

import concourse.bass as bass
import concourse.tile as tile
from concourse import mybir
from concourse.vector_clock import ScopedClock

F32 = mybir.dt.float32
F16 = mybir.dt.float16
AF = mybir.ActivationFunctionType
OP = mybir.AluOpType

C = 128
HEADS = 4
DHEAD = 32
SCALE = DHEAD ** -0.5
EPS = 1e-5
UPS = 4096.0  # fp16-underflow guard: W1T scaled up, undone in GN scales

BIG = 2048  # DMA chunk (columns)
SUB2 = 512  # pass2/3 compute chunk
SUB1 = 128  # pass1 compute chunk
GRP = 4     # pass2/3 chunks per denominator-recip group (= BIG/SUB2)


class TC(tile.TileContext):
    """TileContext whose kernel-tail drain splits sem waits into single-wait
    instructions (this walrus build rejects multi-wait Drain)."""

    def _drain_and_barrier(self, tick_clock, wait_clock):
        nc = self.nc
        drain_inst = nc.sync.drain()
        wait_clock.add_sem_waits(
            drain_inst.ins, ScopedClock({None: tick_clock.global_clock})
        )
        waits = list(drain_inst.ins.sync_info.on_wait)
        if len(waits) > 1:
            drain_inst.ins.sync_info.on_wait.clear()
            num2handle = {h.num: h for h in self.sems.allocated().values()}
            for w in waits:
                nc.sync.wait_ge(num2handle[w.id], w.wait_value)
        nc.all_engine_barrier()
        popped = nc._tile_sem_poison_stack.pop()
        assert popped is self._sem_poison
        nc.clear_and_free_semaphores(list(self.sems.allocated().values()))
        nc.all_engine_barrier()


def build_program(n: int) -> bass.Bass:
    assert n % BIG == 0
    nbig = n // BIG
    nsub2 = n // SUB2
    s1_per_big = BIG // SUB1
    nsub1 = n // SUB1

    nc = bass.Bass()
    xhi = nc.dram_tensor("xhi", [C, n], F16, kind="ExternalInput")
    xlo = nc.dram_tensor("xlo", [C, n], F16, kind="ExternalInput")
    y = nc.dram_tensor("y", [C, n], F32, kind="ExternalOutput")
    wkv = nc.dram_tensor("wkv", [C, 256], F16, kind="ExternalInput")
    wqt = nc.dram_tensor("wqt", [C, C], F16, kind="ExternalInput")
    wots = nc.dram_tensor("wots", [C, C], F32, kind="ExternalInput")
    maskh = nc.dram_tensor("maskh", [C, 32], F16, kind="ExternalInput")
    bmask = nc.dram_tensor("bmask", [C, 4 * C], F16, kind="ExternalInput")
    blockmask = nc.dram_tensor("blockmask", [C, C], F32, kind="ExternalInput")
    ident = nc.dram_tensor("ident", [C, C], F32, kind="ExternalInput")
    onesrow = nc.dram_tensor("onesrow", [1, C], F32, kind="ExternalInput")
    onescol = nc.dram_tensor("onescol", [C, 1], F32, kind="ExternalInput")
    gammarow = nc.dram_tensor("gammarow", [1, C], F32, kind="ExternalInput")
    gammacol = nc.dram_tensor("gammacol", [C, 1], F32, kind="ExternalInput")
    betacol = nc.dram_tensor("betacol", [C, 1], F32, kind="ExternalInput")
    bocol = nc.dram_tensor("bocol", [C, 1], F32, kind="ExternalInput")

    with ExitStack() as top:
        tc = top.enter_context(TC(nc))
        consts = top.enter_context(tc.tile_pool(name="consts", bufs=1))
        xpool = top.enter_context(tc.tile_pool(name="xpool", bufs=3))
        midsb = top.enter_context(tc.tile_pool(name="midsb", bufs=1))

        def cload(name, dram, shape, dt=F32):
            t = consts.tile(shape, dt, name=name)
            nc.sync.dma_start(out=t, in_=dram[:, :])
            return t

        wkv_sb = cload("wkv_sb", wkv, [C, 256], F16)
        wqt_sb = cload("wqt_sb", wqt, [C, C], F16)
        wots_sb = cload("wots_sb", wots, [C, C])
        maskh_sb = cload("maskh_sb", maskh, [C, 32], F16)
        bmask_sb = cload("bmask_sb", bmask, [C, 4 * C], F16)
        blockmask_sb = cload("blockmask_sb", blockmask, [C, C])
        ident_sb = cload("ident_sb", ident, [C, C])
        onesrow_sb = cload("onesrow_sb", onesrow, [1, C])
        onescol_sb = cload("onescol_sb", onescol, [C, 1])
        gammarow_sb = cload("gammarow_sb", gammarow, [1, C])
        gammacol_sb = cload("gammacol_sb", gammacol, [C, 1])
        betacol_sb = cload("betacol_sb", betacol, [C, 1])
        bocol_sb = cload("bocol_sb", bocol, [C, 1])

        # ---------------- pass 1 + mid1 ----------------
        with ExitStack() as p1:
            ctxzpool = p1.enter_context(
                tc.tile_pool(name="ctxzpool", bufs=1, space="PSUM")
            )
            ctxz = ctxzpool.tile([C, 132], F32, name="ctxz")
            with ExitStack() as p1i:
                kvpool = p1i.enter_context(
                    tc.tile_pool(name="kvpool", bufs=3, space="PSUM")
                )
                ekpool = p1i.enter_context(tc.tile_pool(name="ekpool", bufs=3))
                evpool = p1i.enter_context(tc.tile_pool(name="evpool", bufs=4))
                for ci in range(nbig):
                    xt = xpool.tile([C, BIG], F16, name="xt1")
                    nc.sync.dma_start(out=xt, in_=xhi[:, ci * BIG : (ci + 1) * BIG])
                    for sj in range(s1_per_big):
                        j = ci * s1_per_big + sj
                        kv = kvpool.tile([C, 256], F32, name="kv")
                        nc.tensor.matmul(
                            kv,
                            lhsT=xt[:, sj * SUB1 : (sj + 1) * SUB1],
                            rhs=wkv_sb,
                            start=True,
                            stop=True,
                        )
                        ek = ekpool.tile([C, SUB1], F16, name="ek")
                        nc.scalar.activation(ek, kv[:, 0:128], AF.Exp)
                        ev = evpool.tile([C, 132], F16, name="ev")
                        nc.vector.tensor_copy(ev[:, 0:128], kv[:, 128:256])
                        nc.gpsimd.memset(ev[:, 128:132], 1.0)
                        nc.tensor.matmul(
                            ctxz[:, 0:129],
                            lhsT=ek,
                            rhs=ev[:, 0:129],
                            start=(j == 0),
                            stop=(j == nsub1 - 1),
                            skip_group_check=True,
                        )

            # ---------------- mid 1: W1T ----------------
            midps = p1.enter_context(tc.tile_pool(name="midps", bufs=1, space="PSUM"))
            rz_sb = midsb.tile([C, 1], F32, name="rz_sb")
            nc.vector.reciprocal(rz_sb, ctxz[:, 128:129])
            ctxn_sb = midsb.tile([C, C], F32, name="ctxn_sb")
            nc.vector.scalar_tensor_tensor(
                out=ctxn_sb,
                in0=ctxz[:, 0:128],
                scalar=rz_sb,
                in1=blockmask_sb,
                op0=OP.mult,
                op1=OP.mult,
            )
            tpsum = midps.tile([C, C], F32, name="tpsum")
            nc.tensor.transpose(tpsum, ctxn_sb, ident_sb)
            ctxnt_sb = midsb.tile([C, C], F32, name="ctxnt_sb")
            nc.scalar.copy(ctxnt_sb, tpsum)
            w1psum = midps.tile([C, C], F32, name="w1psum")
            nc.tensor.matmul(w1psum, lhsT=ctxnt_sb, rhs=wots_sb, start=True, stop=True)
            w1t_sb = midsb.tile([C, C], F16, name="w1t_sb")
            nc.scalar.copy(w1t_sb, w1psum)

        statspool = top.enter_context(tc.tile_pool(name="statspool", bufs=1))
        qsbuf = statspool.tile([C, nsub2], F32, name="qsbuf")
        sqbuf = statspool.tile([C, nsub2], F32, name="sqbuf")

        # ---------------- pass 2: stats ----------------
        with ExitStack() as p2:
            qppool = p2.enter_context(tc.tile_pool(name="qppool", bufs=2, space="PSUM"))
            dppool = p2.enter_context(tc.tile_pool(name="dppool", bufs=2, space="PSUM"))
            rbpool = p2.enter_context(tc.tile_pool(name="rbpool", bufs=2, space="PSUM"))
            oppool = p2.enter_context(tc.tile_pool(name="oppool", bufs=2, space="PSUM"))
            expqpool = p2.enter_context(tc.tile_pool(name="expqpool", bufs=8))
            rdfpool = p2.enter_context(tc.tile_pool(name="rdfpool", bufs=2))
            rd16pool = p2.enter_context(tc.tile_pool(name="rd16pool", bufs=2))
            qnpool = p2.enter_context(tc.tile_pool(name="qnpool", bufs=3))
            sqscrpool = p2.enter_context(tc.tile_pool(name="sqscrpool", bufs=2))
            for ci in range(nbig):
                xt = xpool.tile([C, BIG], F16, name="xt2")
                nc.sync.dma_start(out=xt, in_=xhi[:, ci * BIG : (ci + 1) * BIG])
                dp4 = dppool.tile([C, SUB2], F32, name="dp4")
                expqs = []
                for m in range(GRP):
                    j = ci * GRP + m
                    xs = xt[:, m * SUB2 : (m + 1) * SUB2]
                    qp = qppool.tile([C, SUB2], F32, name="qp")
                    nc.tensor.matmul(qp, lhsT=wqt_sb, rhs=xs, start=True, stop=True)
                    expq = expqpool.tile([C, SUB2], F16, name="expq")
                    nc.scalar.activation(expq, qp, AF.Exp)
                    expqs.append(expq)
                    nc.tensor.matmul(
                        dp4[32 * m : 32 * m + 32, :],
                        lhsT=maskh_sb,
                        rhs=expq,
                        start=True,
                        stop=True,
                        tile_position=(0, 32 * m),
                    )
                rdf = rdfpool.tile([C, SUB2], F32, name="rdf")
                nc.scalar.activation(rdf, dp4, AF.Ln)
                rd16 = rd16pool.tile([C, SUB2], F16, name="rd16")
                nc.scalar.activation(rd16, rdf, AF.Exp, scale=-1.0)
                for m in range(GRP):
                    j = ci * GRP + m
                    rb = rbpool.tile([C, SUB2], F32, name="rb")
                    nc.tensor.matmul(
                        rb,
                        lhsT=bmask_sb[:, m * C : (m + 1) * C],
                        rhs=rd16,
                        start=True,
                        stop=True,
                    )
                    qn = qnpool.tile([C, SUB2], F16, name="qn")
                    nc.vector.scalar_tensor_tensor(
                        out=qn,
                        in0=expqs[m],
                        scalar=1.0,
                        in1=rb,
                        op0=OP.mult,
                        op1=OP.mult,
                        accum_out=qsbuf[:, j : j + 1],
                    )
                    op = oppool.tile([C, SUB2], F32, name="op")
                    nc.tensor.matmul(op, lhsT=w1t_sb, rhs=qn, start=True, stop=True)
                    sqscr = sqscrpool.tile([C, SUB2], F32, name="sqscr")
                    nc.scalar.activation(
                        sqscr, op, AF.Square, accum_out=sqbuf[:, j : j + 1]
                    )

        # ---------------- mid 2: GN scales ----------------
        NTOT = float(C * n)
        with ExitStack() as m2:
            midps2 = m2.enter_context(tc.tile_pool(name="midps2", bufs=1, space="PSUM"))
            qsumT = midsb.tile([C, 1], F32, name="qsumT")
            nc.vector.reduce_sum(qsumT, qsbuf, axis=mybir.AxisListType.X)
            sqT = midsb.tile([C, 1], F32, name="sqT")
            nc.vector.reduce_sum(sqT, sqbuf, axis=mybir.AxisListType.X)
            qsum16 = midsb.tile([C, 1], F16, name="qsum16")
            nc.vector.tensor_copy(qsum16, qsumT)
            s1psum_ = midps2.tile([C, 1], F32, name="s1psum_")
            nc.tensor.matmul(s1psum_, lhsT=w1t_sb, rhs=qsum16, start=True, stop=True)
            s1col = midsb.tile([C, 1], F32, name="s1col")
            nc.scalar.copy(s1col, s1psum_)
            # bias folding (out' = UPS*out; B' = UPS*bo)
            bo4 = midsb.tile([C, 1], F32, name="bo4")
            nc.vector.tensor_scalar_mul(bo4, bocol_sb, UPS)
            nbo4 = midsb.tile([C, 1], F32, name="nbo4")
            nc.vector.tensor_scalar_mul(nbo4, bo4, float(n))
            combo = midsb.tile([C, 2], F32, name="combo")
            # c0 = S1 + N*B'
            nc.vector.tensor_add(combo[:, 0:1], s1col, nbo4)
            # c1 = S2 + B'*(2*S1 + N*B')
            tt = midsb.tile([C, 1], F32, name="tt")
            nc.vector.scalar_tensor_tensor(
                out=tt, in0=s1col, scalar=2.0, in1=nbo4, op0=OP.mult, op1=OP.add
            )
            nc.vector.scalar_tensor_tensor(
                out=combo[:, 1:2], in0=tt, scalar=bo4, in1=sqT, op0=OP.mult, op1=OP.add
            )
            spsum = midps2.tile([2, 1], F32, name="spsum")
            nc.tensor.matmul(spsum, lhsT=combo, rhs=onescol_sb, start=True, stop=True)
            scol = midsb.tile([2, 1], F32, name="scol")
            nc.scalar.copy(scol, spsum)
            trow = midps2.tile([1, 2], F32, name="trow")
            nc.tensor.matmul(
                trow, lhsT=scol, rhs=ident_sb[0:2, 0:2], start=True, stop=True
            )
            srow = midsb.tile([1, 2], F32, name="srow")
            nc.scalar.copy(srow, trow)
            # M' = T0/NTOT ; E2' = T1/NTOT ; V = (E2' - M'^2)/UPS^2
            mp_sb = midsb.tile([1, 1], F32, name="mp_sb")
            nc.scalar.mul(mp_sb, srow[0:1, 0:1], 1.0 / NTOT)
            msq_sb = midsb.tile([1, 1], F32, name="msq_sb")
            nc.scalar.activation(msq_sb, mp_sb, AF.Square)
            v_sb = midsb.tile([1, 1], F32, name="v_sb")
            nc.vector.scalar_tensor_tensor(
                out=v_sb,
                in0=srow[0:1, 1:2],
                scalar=1.0 / NTOT,
                in1=msq_sb,
                op0=OP.mult,
                op1=OP.subtract,
            )
            vs_sb = midsb.tile([1, 1], F32, name="vs_sb")
            nc.vector.tensor_scalar_mul(vs_sb, v_sb, 1.0 / (UPS * UPS))
            veps_sb = midsb.tile([1, 1], F32, name="veps_sb")
            nc.vector.tensor_scalar_add(veps_sb, vs_sb, EPS)
            l_sb = midsb.tile([1, 1], F32, name="l_sb")
            nc.scalar.activation(l_sb, veps_sb, AF.Ln)
            rstd_sb = midsb.tile([1, 1], F32, name="rstd_sb")
            nc.scalar.activation(rstd_sb, l_sb, AF.Exp, scale=-0.5)
            # M = M'/UPS
            m_sb = midsb.tile([1, 1], F32, name="m_sb")
            nc.scalar.mul(m_sb, mp_sb, 1.0 / UPS)
            mr_sb = midsb.tile([1, 2], F32, name="mr_sb")
            nc.vector.tensor_copy(mr_sb[0:1, 0:1], m_sb)
            nc.vector.tensor_copy(mr_sb[0:1, 1:2], rstd_sb)
            bpsum = midps2.tile([C, 2], F32, name="bpsum")
            nc.tensor.matmul(bpsum, lhsT=onesrow_sb, rhs=mr_sb, start=True, stop=True)
            bcol_sb = midsb.tile([C, 2], F32, name="bcol_sb")
            nc.scalar.copy(bcol_sb, bpsum)
            t_sb = midsb.tile([C, 1], F32, name="t_sb")
            nc.vector.scalar_tensor_tensor(
                out=t_sb,
                in0=bocol_sb,
                scalar=bcol_sb[:, 0:1],
                in1=bcol_sb[:, 1:2],
                op0=OP.subtract,
                op1=OP.mult,
            )
            s2_sb = midsb.tile([C, 1], F32, name="s2_sb")
            nc.vector.scalar_tensor_tensor(
                out=s2_sb,
                in0=t_sb,
                scalar=gammacol_sb,
                in1=betacol_sb,
                op0=OP.mult,
                op1=OP.add,
            )
            # s1row = gammarow * rstd / UPS
            gs_sb = midsb.tile([1, C], F32, name="gs_sb")
            nc.vector.tensor_scalar_mul(gs_sb, gammarow_sb, 1.0 / UPS)
            s1row_sb = midsb.tile([1, C], F32, name="s1row_sb")
            nc.vector.tensor_scalar_mul(s1row_sb, gs_sb, rstd_sb)
            s1psum = midps2.tile([C, C], F32, name="s1psum")
            nc.tensor.matmul(
                s1psum, lhsT=onesrow_sb, rhs=s1row_sb, start=True, stop=True
            )
            w2t_sb = midsb.tile([C, C], F16, name="w2t_sb")
            nc.vector.tensor_mul(w2t_sb, w1t_sb, s1psum)

        # ---------------- pass 3: output ----------------
        with ExitStack() as p3:
            qppool3 = p3.enter_context(
                tc.tile_pool(name="qppool3", bufs=2, space="PSUM")
            )
            dppool3 = p3.enter_context(
                tc.tile_pool(name="dppool3", bufs=2, space="PSUM")
            )
            rbpool3 = p3.enter_context(
                tc.tile_pool(name="rbpool3", bufs=2, space="PSUM")
            )
            oppool3 = p3.enter_context(
                tc.tile_pool(name="oppool3", bufs=2, space="PSUM")
            )
            expqpool3 = p3.enter_context(tc.tile_pool(name="expqpool3", bufs=8))
            rdfpool3 = p3.enter_context(tc.tile_pool(name="rdfpool3", bufs=2))
            rd16pool3 = p3.enter_context(tc.tile_pool(name="rd16pool3", bufs=2))
            qnpool3 = p3.enter_context(tc.tile_pool(name="qnpool3", bufs=3))
            xlopool = p3.enter_context(tc.tile_pool(name="xlopool", bufs=2))
            fpool = p3.enter_context(tc.tile_pool(name="fpool", bufs=2))
            f2pool = p3.enter_context(tc.tile_pool(name="f2pool", bufs=2))
            for ci in range(nbig):
                xt = xpool.tile([C, BIG], F16, name="xt3")
                nc.sync.dma_start(out=xt, in_=xhi[:, ci * BIG : (ci + 1) * BIG])
                xlt = xlopool.tile([C, BIG], F16, name="xlt")
                nc.sync.dma_start(out=xlt, in_=xlo[:, ci * BIG : (ci + 1) * BIG])
                dp4 = dppool3.tile([C, SUB2], F32, name="dp43")
                expqs = []
                for m in range(GRP):
                    xs = xt[:, m * SUB2 : (m + 1) * SUB2]
                    qp = qppool3.tile([C, SUB2], F32, name="qp3")
                    nc.tensor.matmul(qp, lhsT=wqt_sb, rhs=xs, start=True, stop=True)
                    expq = expqpool3.tile([C, SUB2], F16, name="expq3")
                    nc.scalar.activation(expq, qp, AF.Exp)
                    expqs.append(expq)
                    nc.tensor.matmul(
                        dp4[32 * m : 32 * m + 32, :],
                        lhsT=maskh_sb,
                        rhs=expq,
                        start=True,
                        stop=True,
                        tile_position=(0, 32 * m),
                    )
                rdf = rdfpool3.tile([C, SUB2], F32, name="rdf3")
                nc.scalar.activation(rdf, dp4, AF.Ln)
                rd16 = rd16pool3.tile([C, SUB2], F16, name="rd163")
                nc.scalar.activation(rd16, rdf, AF.Exp, scale=-1.0)
                ft = fpool.tile([C, BIG], F32, name="ft")
                for m in range(GRP):
                    xs = xt[:, m * SUB2 : (m + 1) * SUB2]
                    rb = rbpool3.tile([C, SUB2], F32, name="rb3")
                    nc.tensor.matmul(
                        rb,
                        lhsT=bmask_sb[:, m * C : (m + 1) * C],
                        rhs=rd16,
                        start=True,
                        stop=True,
                    )
                    qn = qnpool3.tile([C, SUB2], F16, name="qn3")
                    nc.vector.tensor_mul(qn, expqs[m], rb)
                    op = oppool3.tile([C, SUB2], F32, name="op3")
                    nc.tensor.matmul(op, lhsT=w2t_sb, rhs=qn, start=True, stop=True)
                    nc.vector.scalar_tensor_tensor(
                        out=ft[:, m * SUB2 : (m + 1) * SUB2],
                        in0=op,
                        scalar=s2_sb,
                        in1=xs,
                        op0=OP.add,
                        op1=OP.add,
                    )
                ft2 = f2pool.tile([C, BIG], F32, name="ft2")
                nc.gpsimd.tensor_add(ft2, ft, xlt)
                nc.sync.dma_start(out=y[:, ci * BIG : (ci + 1) * BIG], in_=ft2)

    import bass_rust as _bass_rust

    _bass_rust.generate_event_semaphores(nc)
    return nc


def make_consts(Wq, Wk, Wv, Wo, bo, gn_gamma, gn_beta):
    f = np.float32
    h = np.float16
    wkv = np.concatenate([Wk.T, Wv.T], axis=1)
    d = np.arange(C)
    maskh = (d[:, None] // DHEAD == (np.arange(32) % HEADS)[None, :]).astype(h)
    # bmask[:, m*C + c] = 1 iff partition p == 32*m + c//DHEAD
    bm = np.zeros((C, 4 * C), dtype=h)
    for m in range(4):
        for c in range(C):
            bm[32 * m + c // DHEAD, m * C + c] = 1.0
    blockmask = (d[:, None] // DHEAD == d[None, :] // DHEAD).astype(f)
    return {
        "wkv": np.ascontiguousarray(wkv.astype(h)),
        "wqt": np.ascontiguousarray(Wq.T.astype(h)),
        "wots": np.ascontiguousarray((Wo.T * SCALE * UPS).astype(f)),
        "maskh": np.ascontiguousarray(maskh),
        "bmask": np.ascontiguousarray(bm),
        "blockmask": np.ascontiguousarray(blockmask),
        "ident": np.eye(C, dtype=f),
        "onesrow": np.ones((1, C), dtype=f),
        "onescol": np.ones((C, 1), dtype=f),
        "gammarow": np.ascontiguousarray(gn_gamma.reshape(1, C).astype(f)),
        "gammacol": np.ascontiguousarray(gn_gamma.reshape(C, 1).astype(f)),
        "betacol": np.ascontiguousarray(gn_beta.reshape(C, 1).astype(f)),
        "bocol": np.ascontiguousarray(bo.reshape(C, 1).astype(f)),
    }


@functools.lru_cache(maxsize=2)
def _get_program(n):
    return build_program(n)


def run_on_cores(xf, consts, n, trace=False, tmpdir=None):
    """xf: [B, C, n] fp32. Returns ([B, C, n] fp32, BassKernelResults)."""
    from concourse.bass_utils import run_bass_kernel_spmd

    nc = _get_program(n)
    B = xf.shape[0]
    in_maps = []
    for b in range(B):
        x32 = np.ascontiguousarray(xf[b], dtype=np.float32)
        xh = x32.astype(np.float16)
        xl = (x32 - xh.astype(np.float32)).astype(np.float16)
        in_maps.append({"xhi": xh, "xlo": xl, **consts})
    kw = {}
    if tmpdir is not None:
        kw["tmpdir"] = tmpdir
    res = run_bass_kernel_spmd(nc, in_maps, core_ids=list(range(B)), trace=trace, **kw)
    out = np.stack([res.results[b]["y"] for b in range(B)])
    return out, res


def kernel(x, Wq, Wk, Wv, Wo, bo, gn_gamma, gn_beta):
    x = np.asarray(x, dtype=np.float32)
    B, c, H, W = x.shape
    n = H * W
    consts = make_consts(
        np.asarray(Wq), np.asarray(Wk), np.asarray(Wv), np.asarray(Wo),
        np.asarray(bo), np.asarray(gn_gamma), np.asarray(gn_beta),
    )
    xf = x.reshape(B, c, n)
    out, _ = run_on_cores(xf, consts, n)
    return out.reshape(B, c, H, W)



# revision 2
# speedup vs baseline: 1.0060x; 1.0060x over previous
"""CrossLinearAttention Trainium2 kernel (v2: SBUF-resident x, t-replay).

Per-core: one batch sample (data-parallel over B=8 across 8 NeuronCores).
Per sample x_b: [C=128, N=65536] fp32, shipped as fp16 (rel tolerance 2e-2
makes the hi/lo split unnecessary).

Math (matches the reference, re-associated for TRN2):
  q,k,v = W{q,k,v} @ x ; q softmaxed over d_head (32-groups), k over n.
  ctx_h = softmax_k_h @ v_h^T  (32x32/head) ; out2 = ctx^T q * SCALE
  out = Wo@out2 + bo ; GroupNorm(1 group) over (C,N) ; *gamma+beta ; +x

Structure:
  x is DMA'd once into a resident SBUF tile (16MB of the 26MB SBUF).
  pass1: per 512-col chunk, kv = x^T @ [Wk^T|Wv^T] (4 matmuls into one
         2-bank PSUM tile); one batched exp -> ek fp16; one batched copy
         -> ev fp16 (with a ones column per 128-block); PSUM ctxz
         accumulates ek^T @ [ev|1] -> E V^T (128x128) and Z.
  mid1 : W1T = (ctx/Z .* blockmask)^T @ (Wo^T*SCALE*UPS)  (fp16)
  pass2: per 512-col chunk: qp=Wq@x ; expq=exp(qp) ; head-denoms packed
         4-chunks/PSUM-tile; recip = exp(-ln(D)) ; rb = bcast_mm(recip);
         qn = expq*rb ; op = W1T^T @ qn ; t = op*(TS/UPS)+bo*TS stored
         fp16 to HBM scratch; running sums of t and t^2 via accum_out.
  mid2 : GroupNorm mean/var directly from sum(t), sum(t^2);
         a = gamma*rstd_t, c = beta - a*mu_t  (per-channel columns).
  pass3: stream t back: y = a*t + c + x  (one fused DVE op per 2048 cols),
         store fp16; host casts to fp32.
"""

import sys

sys.path.insert(0, "/opt/trn_rl_repo")

import functools
from contextlib import ExitStack

import numpy as np

import concourse.bass as bass
import concourse.tile as tile
from concourse import mybir
from concourse.vector_clock import ScopedClock

F32 = mybir.dt.float32
F16 = mybir.dt.float16
AF = mybir.ActivationFunctionType
OP = mybir.AluOpType

C = 128
HEADS = 4
DHEAD = 32
SCALE = DHEAD ** -0.5
EPS = 1e-5
UPS = 4096.0  # fp16-underflow guard on W1T (undone in t scaling)
TS = 64.0     # storage scale for t = TS*(out'+bo)

BIG = 2048  # DMA chunk (columns)
SUB = 512   # compute chunk

USE_CUSTOM_DVE = True


class TC(tile.TileContext):
    """TileContext whose kernel-tail drain splits sem waits into single-wait
    instructions (this walrus build rejects multi-wait Drain)."""

    def _drain_and_barrier(self, tick_clock, wait_clock):
        nc = self.nc
        drain_inst = nc.sync.drain()
        wait_clock.add_sem_waits(
            drain_inst.ins, ScopedClock({None: tick_clock.global_clock})
        )
        waits = list(drain_inst.ins.sync_info.on_wait)
        if len(waits) > 1:
            drain_inst.ins.sync_info.on_wait.clear()
            num2handle = {h.num: h for h in self.sems.allocated().values()}
            for w in waits:
                nc.sync.wait_ge(num2handle[w.id], w.wait_value)
        nc.all_engine_barrier()
        popped = nc._tile_sem_poison_stack.pop()
        assert popped is self._sem_poison
        nc.clear_and_free_semaphores(list(self.sems.allocated().values()))
        nc.all_engine_barrier()


def build_program(n: int) -> bass.Bass:
    assert n % BIG == 0
    nbig = n // BIG
    nsub = n // SUB
    sub_per_big = BIG // SUB

    nc = bass.Bass()
    xhi = nc.dram_tensor("xhi", [C, n], F16, kind="ExternalInput")
    y = nc.dram_tensor("y", [C, n], F16, kind="ExternalOutput")
    tdram = nc.dram_tensor("tscratch", [C, n], F16, kind="Internal")
    wkv = nc.dram_tensor("wkv", [C, 256], F16, kind="ExternalInput")
    wqt = nc.dram_tensor("wqt", [C, C], F16, kind="ExternalInput")
    wots = nc.dram_tensor("wots", [C, C], F32, kind="ExternalInput")
    maskh = nc.dram_tensor("maskh", [C, 32], F16, kind="ExternalInput")
    bmask = nc.dram_tensor("bmask", [C, 4 * C], F16, kind="ExternalInput")
    blockmask = nc.dram_tensor("blockmask", [C, C], F32, kind="ExternalInput")
    ident = nc.dram_tensor("ident", [C, C], F32, kind="ExternalInput")
    onesrow = nc.dram_tensor("onesrow", [1, C], F32, kind="ExternalInput")
    onescol = nc.dram_tensor("onescol", [C, 1], F32, kind="ExternalInput")
    gammacol = nc.dram_tensor("gammacol", [C, 1], F32, kind="ExternalInput")
    betacol = nc.dram_tensor("betacol", [C, 1], F32, kind="ExternalInput")
    bocol = nc.dram_tensor("bocol", [C, 1], F32, kind="ExternalInput")
    bos512 = nc.dram_tensor("bos512", [C, SUB], F16, kind="ExternalInput")

    with ExitStack() as top:
        tc = top.enter_context(TC(nc))
        consts = top.enter_context(tc.tile_pool(name="consts", bufs=1))
        xres = top.enter_context(tc.tile_pool(name="xres", bufs=1))
        midsb = top.enter_context(tc.tile_pool(name="midsb", bufs=1))
        statsp = top.enter_context(tc.tile_pool(name="statsp", bufs=1))

        def cload(name, dram, shape, dt=F32):
            t = consts.tile(shape, dt, name=name)
            nc.sync.dma_start(out=t, in_=dram[:, :])
            return t

        wkv_sb = cload("wkv_sb", wkv, [C, 256], F16)
        wqt_sb = cload("wqt_sb", wqt, [C, C], F16)
        wots_sb = cload("wots_sb", wots, [C, C])
        maskh_sb = cload("maskh_sb", maskh, [C, 32], F16)
        bmask_sb = cload("bmask_sb", bmask, [C, 4 * C], F16)
        blockmask_sb = cload("blockmask_sb", blockmask, [C, C])
        ident_sb = cload("ident_sb", ident, [C, C])
        onesrow_sb = cload("onesrow_sb", onesrow, [1, C])
        onescol_sb = cload("onescol_sb", onescol, [C, 1])
        gammacol_sb = cload("gammacol_sb", gammacol, [C, 1])
        betacol_sb = cload("betacol_sb", betacol, [C, 1])
        bocol_sb = cload("bocol_sb", bocol, [C, 1])
        bos512_sb = cload("bos512_sb", bos512, [C, SUB], F16)

        xbig = xres.tile([C, n], F16, name="xbig")
        for ci in range(nbig):
            nc.sync.dma_start(
                out=xbig[:, ci * BIG : (ci + 1) * BIG],
                in_=xhi[:, ci * BIG : (ci + 1) * BIG],
            )

        # ---------------- pass 1 + mid1 ----------------
        with ExitStack() as p1:
            ctxpool = p1.enter_context(
                tc.tile_pool(name="ctxpool", bufs=1, space="PSUM")
            )
            ctxz = ctxpool.tile([C, 132], F32, name="ctxz")
            with ExitStack() as p1i:
                kvpool = p1i.enter_context(
                    tc.tile_pool(name="kvpool", bufs=2, space="PSUM")
                )
                ekpool = p1i.enter_context(tc.tile_pool(name="ekpool", bufs=3))
                evpool = p1i.enter_context(tc.tile_pool(name="evpool", bufs=3))
                for ci in range(nsub):
                    base = ci * SUB
                    kvb = kvpool.tile([C, 4, 256], F32, name="kvb")
                    for j in range(4):
                        nc.tensor.matmul(
                            kvb[:, j : j + 1, :],
                            lhsT=xbig[:, base + 128 * j : base + 128 * (j + 1)],
                            rhs=wkv_sb,
                            start=True,
                            stop=True,
                        )
                    ek = ekpool.tile([C, 4, 128], F16, name="ek")
                    nc.scalar.activation(ek, kvb[:, :, 0:128], AF.Exp)
                    ev = evpool.tile([C, 4, 136], F16, name="ev")
                    nc.vector.tensor_copy(ev[:, :, 0:128], kvb[:, :, 128:256])
                    nc.gpsimd.memset(ev[:, :, 128:129], 1.0)
                    for j in range(4):
                        nc.tensor.matmul(
                            ctxz[:, 0:129],
                            lhsT=ek[:, j : j + 1, :],
                            rhs=ev[:, j : j + 1, 0:129],
                            start=(ci == 0 and j == 0),
                            stop=(ci == nsub - 1 and j == 3),
                            skip_group_check=True,
                        )

            # ---------------- mid 1: W1T ----------------
            midps = p1.enter_context(tc.tile_pool(name="midps", bufs=1, space="PSUM"))
            rz_sb = midsb.tile([C, 1], F32, name="rz_sb")
            nc.vector.reciprocal(rz_sb, ctxz[:, 128:129])
            ctxn_sb = midsb.tile([C, C], F32, name="ctxn_sb")
            nc.vector.scalar_tensor_tensor(
                out=ctxn_sb,
                in0=ctxz[:, 0:128],
                scalar=rz_sb,
                in1=blockmask_sb,
                op0=OP.mult,
                op1=OP.mult,
            )
            tpsum = midps.tile([C, C], F32, name="tpsum")
            nc.tensor.transpose(tpsum, ctxn_sb, ident_sb)
            ctxnt_sb = midsb.tile([C, C], F32, name="ctxnt_sb")
            nc.scalar.copy(ctxnt_sb, tpsum)
            w1psum = midps.tile([C, C], F32, name="w1psum")
            nc.tensor.matmul(w1psum, lhsT=ctxnt_sb, rhs=wots_sb, start=True, stop=True)
            w1t_sb = midsb.tile([C, C], F16, name="w1t_sb")
            nc.scalar.copy(w1t_sb, w1psum)

        # bias column for t: bo * TS
        bo_s = midsb.tile([C, 1], F32, name="bo_s")
        nc.vector.tensor_scalar_mul(bo_s, bocol_sb, TS)

        tsumbuf = statsp.tile([C, nsub], F32, name="tsumbuf")
        t2buf = statsp.tile([C, nsub], F32, name="t2buf")

        # ---------------- pass 2: q pipeline + t + stats ----------------
        with ExitStack() as p2:
            qppool = p2.enter_context(tc.tile_pool(name="qppool", bufs=2, space="PSUM"))
            dppool = p2.enter_context(tc.tile_pool(name="dppool", bufs=2, space="PSUM"))
            rbpool = p2.enter_context(tc.tile_pool(name="rbpool", bufs=2, space="PSUM"))
            oppool = p2.enter_context(tc.tile_pool(name="oppool", bufs=2, space="PSUM"))
            expqpool = p2.enter_context(tc.tile_pool(name="expqpool", bufs=6))
            rdfpool = p2.enter_context(tc.tile_pool(name="rdfpool", bufs=2))
            rd16pool = p2.enter_context(tc.tile_pool(name="rd16pool", bufs=2))
            qnpool = p2.enter_context(tc.tile_pool(name="qnpool", bufs=3))
            tpool = p2.enter_context(tc.tile_pool(name="tpool", bufs=2))
            sqpool = p2.enter_context(tc.tile_pool(name="sqpool", bufs=2))
            for ci in range(nbig):
                tbig = tpool.tile([C, BIG], F16, name="tbig")
                dp4 = dppool.tile([C, SUB], F32, name="dp4")
                expqs = []
                for m in range(sub_per_big):
                    xs = xbig[:, ci * BIG + m * SUB : ci * BIG + (m + 1) * SUB]
                    qp = qppool.tile([C, SUB], F32, name="qp")
                    nc.tensor.matmul(qp, lhsT=wqt_sb, rhs=xs, start=True, stop=True)
                    expq = expqpool.tile([C, SUB], F16, name="expq")
                    nc.scalar.activation(expq, qp, AF.Exp)
                    expqs.append(expq)
                    nc.tensor.matmul(
                        dp4[32 * m : 32 * m + 32, :],
                        lhsT=maskh_sb,
                        rhs=expq,
                        start=True,
                        stop=True,
                        tile_position=(0, 32 * m),
                    )
                rdf = rdfpool.tile([C, SUB], F32, name="rdf")
                nc.scalar.activation(rdf, dp4, AF.Ln)
                rd16 = rd16pool.tile([C, SUB], F16, name="rd16")
                nc.scalar.activation(rd16, rdf, AF.Exp, scale=-1.0)
                for m in range(sub_per_big):
                    j = ci * sub_per_big + m
                    rb = rbpool.tile([C, SUB], F32, name="rb")
                    nc.tensor.matmul(
                        rb,
                        lhsT=bmask_sb[:, m * C : (m + 1) * C],
                        rhs=rd16,
                        start=True,
                        stop=True,
                    )
                    qn = qnpool.tile([C, SUB], F16, name="qn")
                    nc.vector.tensor_mul(qn, expqs[m], rb)
                    op = oppool.tile([C, SUB], F32, name="op")
                    nc.tensor.matmul(op, lhsT=w1t_sb, rhs=qn, start=True, stop=True)
                    tm = tbig[:, m * SUB : (m + 1) * SUB]
                    if m in (0, 2):  # t on ACT
                        nc.scalar.activation(
                            tm,
                            op,
                            AF.Identity,
                            bias=bo_s,
                            scale=TS / UPS,
                            accum_out=tsumbuf[:, j : j + 1],
                        )
                    else:  # t on DVE
                        nc.vector.scalar_tensor_tensor(
                            out=tm,
                            in0=op,
                            scalar=TS / UPS,
                            in1=bos512_sb,
                            op0=OP.mult,
                            op1=OP.add,
                            accum_out=tsumbuf[:, j : j + 1],
                        )
                    if m == 1:  # t^2 on ACT straight from op
                        sqs = sqpool.tile([C, SUB], F32, name="sqs")
                        nc.scalar.activation(
                            sqs,
                            op,
                            AF.Square,
                            bias=bo_s,
                            scale=TS / UPS,
                            accum_out=t2buf[:, j : j + 1],
                        )
                    else:  # t^2 on DVE from tm
                        sqs16 = sqpool.tile([C, SUB], F16, name="sqs16", tag="sqs16")
                        nc.vector.tensor_tensor_reduce(
                            out=sqs16,
                            in0=tm,
                            in1=tm,
                            scale=1.0,
                            scalar=0.0,
                            op0=OP.mult,
                            op1=OP.add,
                            accum_out=t2buf[:, j : j + 1],
                        )
                nc.sync.dma_start(out=tdram[:, ci * BIG : (ci + 1) * BIG], in_=tbig)

        # ---------------- mid 2: GN scales ----------------
        NTOT = float(C * n)
        with ExitStack() as m2:
            midps2 = m2.enter_context(tc.tile_pool(name="midps2", bufs=1, space="PSUM"))
            ts_col = midsb.tile([C, 1], F32, name="ts_col")
            nc.vector.reduce_sum(ts_col, tsumbuf, axis=mybir.AxisListType.X)
            t2_col = midsb.tile([C, 1], F32, name="t2_col")
            nc.vector.reduce_sum(t2_col, t2buf, axis=mybir.AxisListType.X)
            combo = midsb.tile([C, 2], F32, name="combo")
            nc.vector.tensor_copy(combo[:, 0:1], ts_col)
            nc.vector.tensor_copy(combo[:, 1:2], t2_col)
            spsum = midps2.tile([2, 1], F32, name="spsum")
            nc.tensor.matmul(spsum, lhsT=combo, rhs=onescol_sb, start=True, stop=True)
            scol = midsb.tile([2, 1], F32, name="scol")
            nc.scalar.copy(scol, spsum)
            trow = midps2.tile([1, 2], F32, name="trow")
            nc.tensor.matmul(
                trow, lhsT=scol, rhs=ident_sb[0:2, 0:2], start=True, stop=True
            )
            srow = midsb.tile([1, 2], F32, name="srow")
            nc.scalar.copy(srow, trow)
            # neg_mu = -T0/NTOT ; e2 = T1/NTOT ; var = e2 - mu^2
            neg_mu = midsb.tile([1, 1], F32, name="neg_mu")
            nc.scalar.mul(neg_mu, srow[0:1, 0:1], -1.0 / NTOT)
            mu_sq = midsb.tile([1, 1], F32, name="mu_sq")
            nc.scalar.activation(mu_sq, neg_mu, AF.Square)
            veps = midsb.tile([1, 1], F32, name="veps")
            nc.vector.scalar_tensor_tensor(
                out=veps,
                in0=srow[0:1, 1:2],
                scalar=1.0 / NTOT,
                in1=mu_sq,
                op0=OP.mult,
                op1=OP.subtract,
            )
            vepse = midsb.tile([1, 1], F32, name="vepse")
            nc.vector.tensor_scalar_add(vepse, veps, TS * TS * EPS)
            l_sb = midsb.tile([1, 1], F32, name="l_sb")
            nc.scalar.activation(l_sb, vepse, AF.Ln)
            rstd_sb = midsb.tile([1, 1], F32, name="rstd_sb")
            nc.scalar.activation(rstd_sb, l_sb, AF.Exp, scale=-0.5)
            mr = midsb.tile([1, 2], F32, name="mr")
            nc.vector.tensor_copy(mr[0:1, 0:1], neg_mu)
            nc.vector.tensor_copy(mr[0:1, 1:2], rstd_sb)
            bps = midps2.tile([C, 2], F32, name="bps")
            nc.tensor.matmul(bps, lhsT=onesrow_sb, rhs=mr, start=True, stop=True)
            bcol = midsb.tile([C, 2], F32, name="bcol")
            nc.scalar.copy(bcol, bps)
            a_col = midsb.tile([C, 1], F32, name="a_col")
            nc.vector.tensor_mul(a_col, gammacol_sb, bcol[:, 1:2])
            c_col = midsb.tile([C, 1], F32, name="c_col")
            nc.vector.scalar_tensor_tensor(
                out=c_col,
                in0=a_col,
                scalar=bcol[:, 0:1],
                in1=betacol_sb,
                op0=OP.mult,
                op1=OP.add,
            )

        # ---------------- pass 3: y = a*t + c + x ----------------
        with ExitStack() as p3:
            tinpool = p3.enter_context(tc.tile_pool(name="tinpool", bufs=3))
            ypool = p3.enter_context(tc.tile_pool(name="ypool", bufs=3))
            for ci in range(nbig):
                tin = tinpool.tile([C, BIG], F16, name="tin")
                nc.sync.dma_start(out=tin, in_=tdram[:, ci * BIG : (ci + 1) * BIG])
                yb = ypool.tile([C, BIG], F16, name="yb")
                if USE_CUSTOM_DVE:
                    nc.vector.affine_then_add(
                        out=yb,
                        in0=tin,
                        in1=xbig[:, ci * BIG : (ci + 1) * BIG],
                        scale=a_col,
                        bias=c_col,
                    )
                else:
                    nc.vector.scalar_tensor_tensor(
                        out=yb,
                        in0=tin,
                        scalar=a_col,
                        in1=xbig[:, ci * BIG : (ci + 1) * BIG],
                        op0=OP.mult,
                        op1=OP.add,
                    )
                    nc.vector.tensor_scalar_add(yb, yb, c_col)
                nc.sync.dma_start(out=y[:, ci * BIG : (ci + 1) * BIG], in_=yb)

    import bass_rust as _bass_rust

    _bass_rust.generate_event_semaphores(nc)
    return nc


def make_consts(Wq, Wk, Wv, Wo, bo, gn_gamma, gn_beta):
    f = np.float32
    h = np.float16
    Wq, Wk, Wv, Wo = (np.asarray(a, dtype=f) for a in (Wq, Wk, Wv, Wo))
    bo = np.asarray(bo, dtype=f)
    gn_gamma = np.asarray(gn_gamma, dtype=f)
    gn_beta = np.asarray(gn_beta, dtype=f)
    wkv = np.concatenate([Wk.T, Wv.T], axis=1)
    d = np.arange(C)
    maskh = (d[:, None] // DHEAD == (np.arange(32) % HEADS)[None, :]).astype(h)
    bm = np.zeros((C, 4 * C), dtype=h)
    for m in range(4):
        for c in range(C):
            bm[32 * m + c // DHEAD, m * C + c] = 1.0
    blockmask = (d[:, None] // DHEAD == d[None, :] // DHEAD).astype(f)
    return {
        "wkv": np.ascontiguousarray(wkv.astype(h)),
        "wqt": np.ascontiguousarray(Wq.T.astype(h)),
        "wots": np.ascontiguousarray((Wo.T * SCALE * UPS).astype(f)),
        "maskh": np.ascontiguousarray(maskh),
        "bmask": np.ascontiguousarray(bm),
        "blockmask": np.ascontiguousarray(blockmask),
        "ident": np.eye(C, dtype=f),
        "onesrow": np.ones((1, C), dtype=f),
        "onescol": np.ones((C, 1), dtype=f),
        "gammacol": np.ascontiguousarray(gn_gamma.reshape(C, 1)),
        "betacol": np.ascontiguousarray(gn_beta.reshape(C, 1)),
        "bocol": np.ascontiguousarray(bo.reshape(C, 1)),
        "bos512": np.ascontiguousarray(
            np.broadcast_to((bo * TS).reshape(C, 1), (C, SUB)).astype(h)
        ),
    }


@functools.lru_cache(maxsize=2)
def _get_program(n):
    return build_program(n)


def run_on_cores(xf, consts, n, trace=False, tmpdir=None):
    """xf: [B, C, n] fp32. Returns ([B, C, n] fp32, BassKernelResults)."""
    from concourse.bass_utils import run_bass_kernel_spmd

    nc = _get_program(n)
    B = xf.shape[0]
    in_maps = []
    for b in range(B):
        xh = np.ascontiguousarray(xf[b]).astype(np.float16)
        in_maps.append({"xhi": xh, **consts})
    kw = {}
    if tmpdir is not None:
        kw["tmpdir"] = tmpdir
    res = run_bass_kernel_spmd(nc, in_maps, core_ids=list(range(B)), trace=trace, **kw)
    out = np.stack([res.results[b]["y"].astype(np.float32) for b in range(B)])
    return out, res


def kernel(x, Wq, Wk, Wv, Wo, bo, gn_gamma, gn_beta):
    x = np.asarray(x, dtype=np.float32)
    B, c, H, W = x.shape
    n = H * W
    consts = make_consts(Wq, Wk, Wv, Wo, bo, gn_gamma, gn_beta)
    xf = x.reshape(B, c, n)
    out, _ = run_on_cores(xf, consts, n)
    return out.reshape(B, c, H, W)
